# revision 15
# baseline (speedup 1.0000x reference)
"""2D Haar DWT (single level) on Trainium2, 8 NeuronCores, pure data parallel.

Math: with Haar filters + symmetric pad + odd-phase downsample, the DWT
reduces to per-2x2-block butterflies over the input image x:
  ll = 0.5*(x00 + x01 + x10 + x11)   (top-left quadrant of output)
  lh = 0.5*(x00 + x01 - x10 - x11)   (bottom-left)
  hl = 0.5*(x00 - x01 + x10 - x11)   (top-right)
  hh = 0.5*(x00 - x01 - x10 + x11)   (bottom-right)

Processing unit: an image PAIR -> 2 MiB contiguous DMAs (better sustained
HBM rate than 1 MiB).  X[128, 4096] f32: partition p holds rows 4p..4p+3
of both images; in-DMA on the SP HWDGE ring.  Width-pass pair SUMS via
one DVE tensor_reduce reading X sequentially (DVE pays ~3x for strided
reads — avoid); width-pass pair DIFFS on GpSimd with strided reads
(software engine, stride-insensitive).  Both write bf16 T (rel-err
budget 2e-2; bf16 keeps the height pass in DVE 2x mode).  Height pass:
4 pair-wide 2-level-AP bf16 adds/subs on DVE into Yb.  One ACT ACTIVATE
applies the 0.5 scale AND casts bf16->f32 into Y; 2 MiB out-DMA on the
ACT HWDGE ring (separate ring from SP avoids load/store head-of-line
blocking).  Per image: Y[p, c*1024 + q*512 + w] = out[c*256 + 2p + q, w].
"""

import numpy as np

import concourse.mybir as mybir
from concourse import bacc, tile
from concourse.bass_utils import run_bass_kernel_spmd

N_CORES = 8
BATCH = 64
B_PER = BATCH // N_CORES  # 8 images per core
H = W = 512

_nc_cache = None


def build_bass():
    f32 = mybir.dt.float32
    bf16 = mybir.dt.bfloat16
    nc = bacc.Bacc(
        "TRN2", target_bir_lowering=False, debug=False, num_devices=N_CORES
    )
    inp = nc.dram_tensor("inputs", [B_PER, H, W], f32, kind="ExternalInput").ap()
    out = nc.dram_tensor("out", [B_PER, H, W], f32, kind="ExternalOutput").ap()

    with tile.TileContext(nc) as tc:
        with tc.tile_pool(name="p", bufs=3) as pool:
            for i in range(0, B_PER, 2):
                X = pool.tile([128, 4096], f32, tag="X", bufs=5)
                nc.sync.dma_start(
                    out=X[:],
                    in_=inp[i : i + 2].rearrange("j (p r) w -> p j r w", p=128),
                )

                # width pass (unscaled): per image j, T[:, j*2048+0:1024] =
                # pair sums (r-blocks of 256), T[:, j*2048+1024:2048] = diffs
                T = pool.tile([128, 4096], bf16, tag="T")
                with nc.allow_low_precision(reason="bf16 DWT intermediates"):
                    nc.vector.tensor_reduce(
                        out=T[:].rearrange("p (j d x) -> p j d x", j=2, d=2)[
                            :, :, 0, :
                        ],
                        in_=X[:].rearrange(
                            "p (j r k t) -> p (j r) k t", j=2, r=4, t=2
                        ),
                        axis=mybir.AxisListType.X,
                        op=mybir.AluOpType.add,
                    )
                for j in range(2):
                    for r in range(4):
                        o = j * 2048
                        nc.gpsimd.tensor_sub(
                            out=T[:, o + 1024 + r * 256 : o + 1024 + (r + 1) * 256],
                            in0=X[:, o + r * 512 : o + (r + 1) * 512 : 2],
                            in1=X[:, o + r * 512 + 1 : o + (r + 1) * 512 : 2],
                        )

                # height pass (unscaled, bf16 2x mode on DVE), pair-wide ops
                # T view [p, img, sum/dif, q, r, 256]
                # Yb view [p, img, top/bot, q, l/r, 256]
                Yb = pool.tile([128, 4096], bf16, tag="Yb")
                Tv = T[:].rearrange(
                    "p (j d q r k) -> p j d q r k", j=2, d=2, q=2, r=2
                )
                Yv = Yb[:].rearrange(
                    "p (j c q h k) -> p j c q h k", j=2, c=2, q=2, h=2
                )
                nc.vector.tensor_add(
                    out=Yv[:, :, 0, :, 0, :],
                    in0=Tv[:, :, 0, :, 0, :],
                    in1=Tv[:, :, 0, :, 1, :],
                )
                nc.vector.tensor_sub(
                    out=Yv[:, :, 1, :, 0, :],
                    in0=Tv[:, :, 0, :, 0, :],
                    in1=Tv[:, :, 0, :, 1, :],
                )
                nc.vector.tensor_add(
                    out=Yv[:, :, 0, :, 1, :],
                    in0=Tv[:, :, 1, :, 0, :],
                    in1=Tv[:, :, 1, :, 1, :],
                )
                nc.vector.tensor_sub(
                    out=Yv[:, :, 1, :, 1, :],
                    in0=Tv[:, :, 1, :, 0, :],
                    in1=Tv[:, :, 1, :, 1, :],
                )

                # fused 0.5 scale + bf16->f32 cast on ACT, then out-DMA
                # (ACT ring); per image for finer pipeline granularity
                Y = pool.tile([128, 4096], f32, tag="Y")
                for j in range(2):
                    sl = slice(j * 2048, (j + 1) * 2048)
                    nc.scalar.mul(Y[:, sl], Yb[:, sl], 0.5)
                    nc.scalar.dma_start(
                        out=out[i + j].rearrange("(c p q) w -> p c q w", c=2, q=2),
                        in_=Y[:, sl],
                    )

    nc.compile()
    return nc


def kernel(**inputs):
    global _nc_cache
    x = np.ascontiguousarray(
        np.asarray(inputs["inputs"], dtype=np.float32).reshape(BATCH, H, W)
    )
    if _nc_cache is None:
        _nc_cache = build_bass()
    nc = _nc_cache
    in_maps = [
        {"inputs": x[i * B_PER : (i + 1) * B_PER]} for i in range(N_CORES)
    ]
    res = run_bass_kernel_spmd(nc, in_maps, core_ids=list(range(N_CORES))).results
    out = np.concatenate([res[i]["out"] for i in range(N_CORES)], axis=0)
    return out.reshape(BATCH, H, W, 1)


# revision 16
# speedup vs baseline: 1.0183x; 1.0183x over previous
"""2D Haar DWT (single level) on Trainium2, 8 NeuronCores, pure data parallel.

Math: with Haar filters + symmetric pad + odd-phase downsample, the DWT
reduces to per-2x2-block butterflies over the input image x:
  ll = 0.5*(x00 + x01 + x10 + x11)   (top-left quadrant of output)
  lh = 0.5*(x00 + x01 - x10 - x11)   (bottom-left)
  hl = 0.5*(x00 - x01 + x10 - x11)   (top-right)
  hh = 0.5*(x00 - x01 - x10 + x11)   (bottom-right)

Processing unit: an image PAIR -> 2 MiB contiguous DMAs (better sustained
HBM rate than 1 MiB).  X[128, 4096] f32: partition p holds rows 4p..4p+3
of both images; in-DMA on the SP HWDGE ring.  Width-pass pair SUMS via
one DVE tensor_reduce reading X sequentially (DVE pays ~3x for strided
reads — avoid); width-pass pair DIFFS on GpSimd with strided reads
(software engine, stride-insensitive).  Both write bf16 T (rel-err
budget 2e-2; bf16 keeps the height pass in DVE 2x mode).  Height pass:
4 pair-wide 2-level-AP bf16 adds/subs on DVE into Yb.  One ACT ACTIVATE
applies the 0.5 scale AND casts bf16->f32 into Y; 2 MiB out-DMA on the
ACT HWDGE ring (separate ring from SP avoids load/store head-of-line
blocking).  Per image: Y[p, c*1024 + q*512 + w] = out[c*256 + 2p + q, w].
"""

import numpy as np

import concourse.mybir as mybir
from concourse import bacc, tile
from concourse.bass_utils import run_bass_kernel_spmd

N_CORES = 8
BATCH = 64
B_PER = BATCH // N_CORES  # 8 images per core
H = W = 512

_nc_cache = None


def build_bass():
    f32 = mybir.dt.float32
    bf16 = mybir.dt.bfloat16
    nc = bacc.Bacc(
        "TRN2", target_bir_lowering=False, debug=False, num_devices=N_CORES
    )
    inp = nc.dram_tensor("inputs", [B_PER, H, W], f32, kind="ExternalInput").ap()
    out = nc.dram_tensor("out", [B_PER, H, W], f32, kind="ExternalOutput").ap()

    with tile.TileContext(nc) as tc:
        with tc.tile_pool(name="p", bufs=3) as pool:
            for i in range(0, B_PER, 2):
                X = pool.tile([128, 4096], f32, tag="X", bufs=4)
                nc.sync.dma_start(
                    out=X[:],
                    in_=inp[i : i + 2].rearrange("j (p r) w -> p j r w", p=128),
                )

                # width pass (unscaled): per image j, T[:, j*2048+0:1024] =
                # pair sums (r-blocks of 256), T[:, j*2048+1024:2048] = diffs
                T = pool.tile([128, 4096], bf16, tag="T")
                with nc.allow_low_precision(reason="bf16 DWT intermediates"):
                    nc.vector.tensor_reduce(
                        out=T[:].rearrange("p (j d x) -> p j d x", j=2, d=2)[
                            :, :, 0, :
                        ],
                        in_=X[:].rearrange(
                            "p (j r k t) -> p (j r) k t", j=2, r=4, t=2
                        ),
                        axis=mybir.AxisListType.X,
                        op=mybir.AluOpType.add,
                    )
                for j in range(2):
                    for r in range(4):
                        o = j * 2048
                        nc.gpsimd.tensor_sub(
                            out=T[:, o + 1024 + r * 256 : o + 1024 + (r + 1) * 256],
                            in0=X[:, o + r * 512 : o + (r + 1) * 512 : 2],
                            in1=X[:, o + r * 512 + 1 : o + (r + 1) * 512 : 2],
                        )

                # height pass (unscaled, bf16 2x mode on DVE), pair-wide ops
                # T view [p, img, sum/dif, q, r, 256]
                # Yb view [p, img, top/bot, q, l/r, 256]
                Yb = pool.tile([128, 4096], bf16, tag="Yb")
                Tv = T[:].rearrange(
                    "p (j d q r k) -> p j d q r k", j=2, d=2, q=2, r=2
                )
                Yv = Yb[:].rearrange(
                    "p (j c q h k) -> p j c q h k", j=2, c=2, q=2, h=2
                )
                nc.vector.tensor_add(
                    out=Yv[:, :, 0, :, 0, :],
                    in0=Tv[:, :, 0, :, 0, :],
                    in1=Tv[:, :, 0, :, 1, :],
                )
                nc.vector.tensor_sub(
                    out=Yv[:, :, 1, :, 0, :],
                    in0=Tv[:, :, 0, :, 0, :],
                    in1=Tv[:, :, 0, :, 1, :],
                )
                nc.vector.tensor_add(
                    out=Yv[:, :, 0, :, 1, :],
                    in0=Tv[:, :, 1, :, 0, :],
                    in1=Tv[:, :, 1, :, 1, :],
                )
                nc.vector.tensor_sub(
                    out=Yv[:, :, 1, :, 1, :],
                    in0=Tv[:, :, 1, :, 0, :],
                    in1=Tv[:, :, 1, :, 1, :],
                )

                # fused 0.5 scale + bf16->f32 cast on ACT, then out-DMA
                # (ACT ring); per image for finer pipeline granularity
                Y = pool.tile([128, 4096], f32, tag="Y")
                for j in range(2):
                    sl = slice(j * 2048, (j + 1) * 2048)
                    nc.scalar.mul(Y[:, sl], Yb[:, sl], 0.5)
                    nc.scalar.dma_start(
                        out=out[i + j].rearrange("(c p q) w -> p c q w", c=2, q=2),
                        in_=Y[:, sl],
                    )

    nc.compile()
    return nc


def kernel(**inputs):
    global _nc_cache
    x = np.ascontiguousarray(
        np.asarray(inputs["inputs"], dtype=np.float32).reshape(BATCH, H, W)
    )
    if _nc_cache is None:
        _nc_cache = build_bass()
    nc = _nc_cache
    in_maps = [
        {"inputs": x[i * B_PER : (i + 1) * B_PER]} for i in range(N_CORES)
    ]
    res = run_bass_kernel_spmd(nc, in_maps, core_ids=list(range(N_CORES))).results
    out = np.concatenate([res[i]["out"] for i in range(N_CORES)], axis=0)
    return out.reshape(BATCH, H, W, 1)
